# revision 39
# baseline (speedup 1.0000x reference)
"""Trainium2 Bass kernel for DFlashAttention (draft/target cross-attention).

Sharding: TP=2 over heads (16 q heads / 4 kv heads per core) x DP=4 over batch.
Core c = tp*4 + b. Each core computes a partial output [64, 2048] (its 16 heads
through its slice of Wo); the host sums the two TP partials per batch.

Host staging: activations are transposed and cast to bf16 on the host
(tgtT/hidT with the model dim leading), wq columns are permuted so q-head
blocks land j-major for the score matmuls, wk|wv are concatenated.

Per-core pipeline (single fused streaming loop over 32 ctx tiles of 128):
  - K/V projection from resident X^T chunks (bf16, fp32 PSUM accum).
  - RMS-norm + RoPE on K fully on the DVE; rsqrt via the pow ALU op so the
    Act engine only ever runs Exp (no activation-table thrash). The norm
    weights (x8, absorbing the 1/sqrt(mean) scale) are pre-folded into the
    cos/sin tables.
  - K^T materialized via xbar DMA transposes (no PE transposes anywhere).
  - Scores computed transposed (S^T = K Q^T) so probs come out kv-major and
    feed the PV matmul directly; softmax sums ride along as a ones-column in
    V (row 64 of the PV accumulator); normalization happens once at the end.
  - No max-subtraction in softmax (scores provably bounded, see baseline).
  - Output projection in bf16.
"""

import os
import numpy as np
from contextlib import ExitStack

B, QL, CTX, KV, D = 4, 64, 4096, 4160, 2048
H, KVH, HD = 32, 8, 64
TP, DP = 2, 4
HL, KVHL = H // TP, KVH // TP          # 16 q heads, 4 kv heads per core
GL = KVHL                              # 4 GQA groups per core (4 q heads each)
DCH = D // 128                         # 16 contraction chunks
NT = CTX // 128                        # 32 ctx kv tiles
EPS = 1e-6
# q-head permutation: j-major (j = group%2 selects the partition half shared
# with the kv head), so score rhs slices are contiguous. perm[slot] = orig head.
QPERM = [0, 1, 2, 3, 8, 9, 10, 11, 4, 5, 6, 7, 12, 13, 14, 15]
# wq column order: pair-interleaved so each 128-col chunk c of Qb holds
# (QPERM[c], QPERM[8+c]) and a plain 2D xbar transpose of the chunk stacks
# them at partition halves (0:64, 64:128) = exactly QT2[:, c, :].
QPERM2 = [QPERM[8 * (i % 2) + i // 2] for i in range(16)]
# group g -> (partition half j, kv-slot jj, q-slot block offset in QPERM order)
G_J = [g % 2 for g in range(GL)]
G_JJ = [g // 2 for g in range(GL)]
G_QOFF = [8 * (g % 2) + 4 * (g // 2) for g in range(GL)]

_NC = None
LAST_RESULT = None


def _patch_drain_split():
    """The walrus build in this container rejects >1 sync-wait on a TPB_CTRL
    Drain; split the TileContext final-drain waits across single-wait NOPs."""
    import concourse.tile as tile
    import concourse.mybir as mybir
    from concourse.vector_clock import ScopedClock

    if getattr(tile.TileContext, "_drain_split_patched", False):
        return

    def _drain_and_barrier(self, tick_clock, wait_clock):
        nc = self.nc
        drain_inst = nc.sync.drain()
        wait_clock.add_sem_waits(
            drain_inst.ins, ScopedClock({None: tick_clock.global_clock})
        )
        si = drain_inst.ins.sync_info
        if si is not None and len(si.on_wait) > 1:
            waits = list(si.on_wait)
            si.on_wait = []
            for w in waits:
                n = nc.sync.nop(nofuse=True, hint="drain_wait_split")
                n.ins.sync_info = mybir.SyncInfo(on_wait=[w], on_update=[])
        nc.all_engine_barrier()
        assert self.sems is not None
        popped = nc._tile_sem_poison_stack.pop()
        assert popped is self._sem_poison
        nc.clear_and_free_semaphores(list(self.sems.allocated().values()))
        nc.all_engine_barrier()

    tile.TileContext._drain_and_barrier = _drain_and_barrier
    tile.TileContext._drain_split_patched = True


def _split_excess_waits(nc, mybir, limit=1):
    """The walrus build here allows very few sync-waits per instruction;
    hoist excess waits onto single-wait NOPs on the same engine queue."""
    k = 0
    for f in nc.m.functions:
        for b in f.blocks:
            out = []
            for inst in b.instructions:
                si = inst.sync_info
                if si is not None and len(si.on_wait) > limit:
                    waits = list(si.on_wait)
                    si.on_wait = waits[-limit:]
                    for w in waits[:-limit]:
                        n = mybir.InstEventSemaphore(name=f"I-ws{k}", ins=[], outs=[])
                        k += 1
                        n.engine = inst.engine
                        n.sync_info = mybir.SyncInfo(on_wait=[w], on_update=[])
                        nc.register_instruction(n)
                        out.append(n)
                out.append(inst)
            b.instructions[:] = out


def build_nc():
    import concourse.bass as bass
    import concourse.mybir as mybir
    import concourse.tile as tile

    _patch_drain_split()

    f32 = mybir.dt.float32
    f32r = mybir.dt.float32r
    bf16 = mybir.dt.bfloat16
    MUL = mybir.AluOpType.mult
    ADD = mybir.AluOpType.add
    POW = mybir.AluOpType.pow
    AXX = mybir.AxisListType.X
    EXP = mybir.ActivationFunctionType.Exp
    SQ = mybir.ActivationFunctionType.Square
    LN = mybir.ActivationFunctionType.Ln

    nc = bass.Bass("TRN2", target_bir_lowering=False, debug=False, num_devices=8)

    # all inputs are host-staged into SBUF-native layouts (partition dim
    # first, contiguous per partition) so every DMA is one descriptor per
    # partition — the DMA queues are descriptor-bound, not byte-bound.
    tgtB = nc.dram_tensor("tgtB", [128, 8, DCH, 512], bf16, kind="ExternalInput").ap()
    hidP = nc.dram_tensor("hidP", [128, DCH, QL], bf16, kind="ExternalInput").ap()
    cosP = nc.dram_tensor("cosP", [128, NT, HD], f32, kind="ExternalInput").ap()
    sinP = nc.dram_tensor("sinP", [128, NT, HD], f32, kind="ExternalInput").ap()
    cosd = nc.dram_tensor("cosd", [QL, HD], f32, kind="ExternalInput").ap()
    sind = nc.dram_tensor("sind", [QL, HD], f32, kind="ExternalInput").ap()
    wqP = nc.dram_tensor("wqP", [128, DCH, HL * HD], bf16, kind="ExternalInput").ap()
    wkvP = nc.dram_tensor("wkvP", [128, DCH, 512], bf16, kind="ExternalInput").ap()
    woP = nc.dram_tensor("woP", [QL, HL, D], bf16, kind="ExternalInput").ap()
    qw = nc.dram_tensor("qw", [1, HD], f32, kind="ExternalInput").ap()
    kw = nc.dram_tensor("kw", [1, HD], f32, kind="ExternalInput").ap()
    out = nc.dram_tensor("out", [QL, D], f32, kind="ExternalOutput").ap()

    # The x8 in the folded cos/sin tables cancels exactly against
    # riv = (ssq + 64*eps)^-0.5 = rsqrt(mean + eps)/8, so Qb/Kb hold the
    # exact normed+roped vectors and the score scale is just 1/sqrt(hd).
    ESC = 0.125

    with tile.TileContext(nc) as tc, ExitStack() as ctx:
        const = ctx.enter_context(tc.tile_pool(name="const", bufs=1))
        xt_pool = ctx.enter_context(tc.tile_pool(name="xt", bufs=2))
        wq_pool = ctx.enter_context(tc.tile_pool(name="wqp", bufs=2))
        kb_pool = ctx.enter_context(tc.tile_pool(name="kb", bufs=2))
        rs_pool = ctx.enter_context(tc.tile_pool(name="rs", bufs=2))
        stats = ctx.enter_context(tc.tile_pool(name="stats", bufs=2))
        pt_pool = ctx.enter_context(tc.tile_pool(name="pt", bufs=2))

        # ---- persistent SBUF tensors ----
        WKV = const.tile([128, DCH, 512], bf16)   # [d128, chunk, k(256)|v(256)]
        KT2 = const.tile([128, 2, KV], bf16)      # [hd | 2 stacked kv heads, jj, kv]
        VN4 = const.tile([128, NT + 1, GL, HD + 1], bf16)  # v natural + ones col
        QT2 = const.tile([128, 8, QL], bf16)      # [hd | j half, head slot, q]
        WO = const.tile([QL, HL, D], bf16)        # [hd, head, d]
        HT = const.tile([128, DCH, QL], bf16)     # hidden^T chunks
        COSW = const.tile([128, NT, HD], f32)     # ctx cos * kw * 8
        SINSW = const.tile([128, NT, HD], f32)    # ctx sin pattern * rot(kw) * 8
        CDQ = const.tile([QL, HD], f32)
        SDQ = const.tile([QL, HD], f32)
        CDK = const.tile([QL, HD], f32)
        SDK = const.tile([QL, HD], f32)
        ONES = const.tile([1, 128], f32)
        EPSB = const.tile([128, 1], f32)
        NW = const.tile([128, 4, HD], f32)        # bcast kw8, rot(kw8), qw8, rot(qw8)
        KW1 = const.tile([1, HD], f32)
        QW1 = const.tile([1, HD], f32)
        W8 = const.tile([1, 4, HD], f32)
        Qb = const.tile([QL, HL * HD], bf16)
        Kdb = const.tile([QL, KVHL * HD], bf16)
        OTb = const.tile([QL, HL, HD], bf16)
        OUTT = const.tile([QL, D], f32)

        # ---- DMAs: X^T block 0 first (gates the streaming loop), then the
        # small stuff phase 1 needs ----
        XTs = [xt_pool.tile([128, DCH, 512], bf16, tag="xt", name=f"xt{i}")
               for i in range(2)]

        def load_xt(blk, part=None):
            # part=None loads the whole block; part=k loads quarter k — the
            # steady-state loop issues one quarter per tile so XT transfers
            # never monopolize DMA queues at block boundaries.
            dst = XTs[blk % 2]
            parts = range(4) if part is None else [part]
            for k in parts:
                i = 4 * k
                nc.sync.dma_start(dst[:, i:i + 4, :], tgtB[:, blk, i:i + 4, :])

        nc.sync.dma_start(KW1[:], kw)
        nc.sync.dma_start(QW1[:], qw)
        nc.sync.dma_start(HT[:], hidP)
        load_xt(0)
        for half in range(4):
            sl = slice(half * 4, half * 4 + 4)
            nc.sync.dma_start(WKV[:, sl, :], wkvP[:, sl, :])
        nc.sync.dma_start(COSW[:], cosP)
        nc.sync.dma_start(SINSW[:], sinP)
        nc.sync.dma_start(CDQ[:], cosd)
        nc.sync.dma_start(SDQ[:], sind)
        nc.sync.dma_start(CDK[:], cosd)
        nc.sync.dma_start(SDK[:], sind)

        nc.vector.memset(ONES[:], 1.0)
        nc.vector.memset(EPSB[:], float(HD) * EPS)
        nc.vector.memset(VN4[:, :, :, HD:HD + 1], 1.0)

        # ---- norm-weight broadcast tiles (x8 fold) ----
        # W8 rows: [kw*8, rot(kw*8), qw*8, rot(qw*8)]
        nc.vector.tensor_scalar_mul(W8[:, 0, :], KW1[:], 8.0)
        nc.vector.tensor_scalar_mul(W8[:, 2, :], QW1[:], 8.0)
        for r in (0, 2):
            nc.vector.tensor_copy(W8[:, r + 1, 0:32], W8[:, r, 32:64])
            nc.vector.tensor_copy(W8[:, r + 1, 32:64], W8[:, r, 0:32])

        with tc.tile_pool(name="ps1", bufs=1, space="PSUM") as ps1, \
             tc.tile_pool(name="psq", bufs=1, space="PSUM") as psq:
            ps_nw = ps1.tile([128, 4 * HD], f32, tag="nw")
            nc.tensor.matmul(ps_nw[:], lhsT=ONES[:], rhs=W8[:].rearrange("o f d -> o (f d)"),
                             start=True, stop=True)
            nc.vector.tensor_copy(NW[:].rearrange("p f d -> p (f d)"), ps_nw[:])

            # fold weights into rope tables.
            # cos'[i] = cos[i]*w8[i]; sin'[i<32] = -sin[i]*w8[i+32],
            # sin'[i>=32] = sin[i]*w8[i-32]  (rot already applied in NW rows 1/3)
            nwc = NW[:, 0, None, :].broadcast_to([128, NT, HD])
            nc.vector.tensor_mul(COSW[:], COSW[:], nwc)
            nc.vector.scalar_tensor_tensor(
                out=SINSW[:, :, 0:32], in0=SINSW[:, :, 0:32], scalar=-1.0,
                in1=NW[:, 1, None, 0:32].broadcast_to([128, NT, 32]),
                op0=MUL, op1=MUL,
            )
            nc.vector.tensor_mul(
                SINSW[:, :, 32:64], SINSW[:, :, 32:64],
                NW[:, 1, None, 32:64].broadcast_to([128, NT, 32]),
            )
            for ctile, stile, rw in ((CDK, SDK, 0), (CDQ, SDQ, 2)):
                nc.vector.tensor_mul(ctile[:], ctile[:], NW[0:QL, rw, :])
                nc.vector.scalar_tensor_tensor(
                    out=stile[:, 0:32], in0=stile[:, 0:32], scalar=-1.0,
                    in1=NW[0:QL, rw + 1, 0:32], op0=MUL, op1=MUL,
                )
                nc.vector.tensor_mul(stile[:, 32:64], stile[:, 32:64],
                                     NW[0:QL, rw + 1, 32:64])

            def rms_rope(src3, P, nh, cos_ap, sins_ap, out_bf, tag):
                """src3: [P, nh, 64] view (psum ok). out_bf: [P, nh*64] bf16.
                Computes 8 * rms_norm(x, w) roped, with w*8 folded into
                cos_ap/sins_ap [P, 64]."""
                scr = rs_pool.tile([P, nh * HD], f32, tag="rrs", name=tag + "s")
                scr3 = scr[:].rearrange("p (h d) -> p h d", d=HD)
                rot = rs_pool.tile([P, nh * HD], f32, tag="rrr", name=tag + "r")
                rot3 = rot[:].rearrange("p (h d) -> p h d", d=HD)
                mss = stats.tile([P, nh], f32, tag="rrm", name=tag + "m")
                riv = stats.tile([P, nh], f32, tag="rrv", name=tag + "v")
                # square on Act (in every act table): psum reads same tensor
                # twice are illegal on the DVE
                nc.scalar.activation(scr3[:], src3, SQ)
                nc.vector.reduce_sum(mss[:, :, None], scr3[:], axis=AXX)
                # riv = (ssq + 64*eps)^-0.5 = exp(-0.5*ln(ssq + 64*eps));
                # ln/exp/square/copy share one act table, so the Act engine
                # never reloads tables. (the x8 of 1/sqrt(mean) lives in the
                # cos/sin tables)
                lg = stats.tile([P, nh], f32, tag="rrl", name=tag + "l")
                nc.scalar.activation(lg[:], mss[:], LN, bias=EPSB[0:P, :], scale=1.0)
                nc.scalar.activation(riv[:], lg[:], EXP, scale=-0.5)
                cb = cos_ap[:, None, :].broadcast_to([P, nh, HD])
                s0 = sins_ap[:, None, 0:32].broadcast_to([P, nh, 32])
                s1 = sins_ap[:, None, 32:64].broadcast_to([P, nh, 32])
                nc.vector.tensor_mul(rot3[:, :, 0:32], src3[:, :, 32:64], s0)
                nc.vector.tensor_mul(rot3[:, :, 32:64], src3[:, :, 0:32], s1)
                nc.vector.tensor_mul(scr3[:], src3, cb)
                nc.vector.tensor_add(scr[:], scr[:], rot[:])
                ob3 = out_bf.rearrange("p (h d) -> p h d", d=HD)
                nc.vector.tensor_mul(ob3[:], scr3[:], riv[:, :, None].broadcast_to([P, nh, HD]))

            # ---- phase 1: q + draft k/v ----
            ps_q = [psq.tile([QL, 512], f32, tag=f"q{c}", name=f"ps_q{c}")
                    for c in range(2)]
            ps_kvd = psq.tile([QL, 512], f32, tag="kvd")
            for i in range(DCH):
                if i % 4 == 0:
                    wqt = wq_pool.tile([128, 4, HL * HD], bf16, tag="w")
                    nc.sync.dma_start(wqt[:], wqP[:, i:i + 4, :])
                for c in range(2):
                    nc.tensor.matmul(
                        ps_q[c][:], lhsT=HT[:, i, :],
                        rhs=wqt[:, i % 4, c * 512:(c + 1) * 512],
                        start=(i == 0), stop=(i == DCH - 1),
                    )
                nc.tensor.matmul(
                    ps_kvd[:], lhsT=HT[:, i, :], rhs=WKV[:, i, :],
                    start=(i == 0), stop=(i == DCH - 1),
                )

            for c in range(4):
                rms_rope(ps_q[c // 2][:, (c % 2) * 256:(c % 2) * 256 + 256]
                         .rearrange("p (h d) -> p h d", d=HD), QL, KVHL,
                         CDQ[:], SDQ[:], Qb[:, c * 256:(c + 1) * 256], "q")
            # monolithic 3D-out xbar transposes are broken on HW; per-128-col
            # 2D transposes with the pair-interleaved wq column order land
            # each chunk's head pair at partition halves (0:64, 64:128).
            for cch in range(8):
                nc.sync.dma_start_transpose(
                    QT2[:, cch, :], Qb[:, cch * 128:(cch + 1) * 128]
                )

            rms_rope(ps_kvd[:, 0:256].rearrange("p (h d) -> p h d", d=HD), QL, KVHL,
                     CDK[:], SDK[:], Kdb[:], "kd")
            nc.scalar.copy(
                VN4[0:QL, NT, :, 0:HD],
                ps_kvd[:, 256:512].rearrange("p (h d) -> p h d", d=HD),
            )
            nc.sync.dma_start_transpose(KT2[:, 0, CTX:KV], Kdb[:, 0:128])
            nc.sync.dma_start_transpose(KT2[:, 1, CTX:KV], Kdb[:, 128:256])

        # ---- fused streaming loop over ctx tiles ----
        ps_o_cm = tc.tile_pool(name="pso", bufs=1, space="PSUM")
        ps_kv_cm = tc.tile_pool(name="pskv", bufs=2, space="PSUM")
        ps_s_cm = tc.tile_pool(name="pss", bufs=1, space="PSUM")
        ps_op = ps_o_cm.__enter__()
        ps_kvp = ps_kv_cm.__enter__()
        ps_sp = ps_s_cm.__enter__()
        ps_o = [ps_op.tile([HD + 1, 4 * QL], f32, tag=f"o{g}", name=f"ps_o{g}")
                for g in range(GL)]

        def scores_block(t, cw):
            """Emit scores + exp for kv tile t ([c0, c0+cw))."""
            c0 = t * 128
            pss = []
            for j in range(2):
                ps_s = ps_sp.tile([128, 512], f32, tag=f"s{j}", name=f"ps_s{t}_{j}")
                for a in range(2):
                    g = j + 2 * a
                    nc.tensor.matmul(
                        ps_s[0:cw, a * 256:(a + 1) * 256],
                        lhsT=KT2[64 * j:64 * j + 64, G_JJ[g], c0:c0 + cw],
                        rhs=QT2[64 * j:64 * j + 64, 4 * G_JJ[g]:4 * G_JJ[g] + 4, :],
                        start=True, stop=True,
                    )
                pss.append(ps_s)
            pts = []
            for j in range(2):
                pt = pt_pool.tile([128, 512], bf16, tag=f"p{j}", name=f"pt{t}_{j}")
                nc.scalar.activation(pt[0:cw, :], pss[j][0:cw, :], EXP, scale=ESC)
                pts.append(pt)
            return pts

        def pv_block(t, cw, pts, start, stop):
            for j in range(2):
                for a in range(2):
                    g = j + 2 * a
                    nc.tensor.matmul(
                        ps_o[g][:],
                        lhsT=VN4[0:cw, t, g, :],
                        rhs=pts[j][0:cw, a * 256:(a + 1) * 256],
                        start=start, stop=stop,
                    )

        def rope_k(t):
            ps_kv = kv_tiles[t % 2]
            kb = kb_pool.tile([128, KVHL * HD], bf16, tag="kb", name=f"kb{t}")
            rms_rope(ps_kv[:, 0:256].rearrange("p (h d) -> p h d", d=HD), 128, KVHL,
                     COSW[:, t, :], SINSW[:, t, :], kb[:], "k")
            nc.sync.dma_start_transpose(KT2[:, 0, t * 128:(t + 1) * 128], kb[:, 0:128])
            nc.sync.dma_start_transpose(KT2[:, 1, t * 128:(t + 1) * 128], kb[:, 128:256])
            nc.vector.tensor_copy(
                VN4[:, t, :, 0:HD],
                ps_kv[:, 256:512].rearrange("p (h d) -> p h d", d=HD),
            )

        # scores/PV lag the projection by 3 tiles: the rope chain (Act/DVE,
        # ~3us latency) plus the K^T xbar DMA (~2us init+transfer) must
        # complete before the scores read KT2 — lag 2 stalls the PE every
        # tile and resets its p-state ramp.
        LAG = 3
        kv_tiles = [None, None]
        for t in range(NT):
            if t // 4 + 1 < 8:
                load_xt(t // 4 + 1, part=t % 4)
            if 8 <= t < 16:
                hh = 2 * (t - 8)
                nc.sync.dma_start(WO[:, hh:hh + 2, :], woP[:, hh:hh + 2, :])
            pts = None
            if t >= LAG:
                pts = scores_block(t - LAG, 128)
            ps_kv = ps_kvp.tile([128, 512], f32, tag="kv", name=f"ps_kv{t}")
            kv_tiles[t % 2] = ps_kv
            xt = XTs[(t // 4) % 2]
            for i in range(DCH):
                nc.tensor.matmul(
                    ps_kv[:], lhsT=xt[:, i, (t % 4) * 128:(t % 4) * 128 + 128],
                    rhs=WKV[:, i, :],
                    start=(i == 0), stop=(i == DCH - 1),
                )
            if t >= LAG:
                pv_block(t - LAG, 128, pts, start=(t - LAG == 0), stop=False)
            if t >= 1:
                rope_k(t - 1)

        # drain the pipeline: tiles NT-LAG..NT-1, then the draft tail
        for tt in range(NT - LAG, NT):
            pts = scores_block(tt, 128)
            pv_block(tt, 128, pts, start=False, stop=False)
            if tt == NT - LAG:
                rope_k(NT - 1)
        pts = scores_block(NT, QL)
        pv_block(NT, QL, pts, start=False, stop=True)

        ps_s_cm.__exit__(None, None, None)
        ps_kv_cm.__exit__(None, None, None)

        # ---- normalize attention output ----
        with tc.tile_pool(name="psn", bufs=2, space="PSUM") as psn:
            for g in range(GL):
                li = stats.tile([1, 4 * QL], f32, tag="li", name=f"li{g}", bufs=1)
                nc.vector.reciprocal(li[:], ps_o[g][HD:HD + 1, :])
                ps_lib = psn.tile([HD, 4 * QL], f32, tag="lib", name=f"lib{g}")
                nc.tensor.matmul(ps_lib[:], lhsT=ONES[:, 0:HD], rhs=li[:],
                                 start=True, stop=True)
                lib = stats.tile([HD, 4 * QL], f32, tag="libs", name=f"libs{g}", bufs=1)
                nc.vector.tensor_copy(lib[:], ps_lib[:])
                off = G_QOFF[g]
                nc.vector.tensor_mul(
                    OTb[:, off:off + 4, :],
                    ps_o[g][0:HD, :].rearrange("p (h q) -> p h q", q=QL),
                    lib[:].rearrange("p (h q) -> p h q", q=QL),
                )

        ps_o_cm.__exit__(None, None, None)

        # ---- output projection (bf16) ----
        with tc.tile_pool(name="ps4", bufs=1, space="PSUM") as ps4:
            accs = [ps4.tile([QL, 512], f32, tag=f"a{c}", name=f"acc{c}")
                    for c in range(4)]
            for hp in range(HL):
                h = QPERM[hp]
                for cc in range(4):
                    nc.tensor.matmul(
                        accs[cc][:],
                        lhsT=OTb[:, hp, :],
                        rhs=WO[:, h, cc * 512:(cc + 1) * 512],
                        start=(hp == 0), stop=(hp == HL - 1),
                    )
            for cc in range(4):
                if cc % 2 == 0:
                    nc.vector.tensor_copy(OUTT[:, cc * 512:(cc + 1) * 512], accs[cc][:])
                else:
                    nc.scalar.copy(OUTT[:, cc * 512:(cc + 1) * 512], accs[cc][:])
        nc.sync.dma_start(out, OUTT[:])

    _split_excess_waits(nc, mybir)
    return nc


def _get_nc():
    global _NC
    if _NC is None:
        _NC = build_nc()
    return _NC


def make_in_maps(hidden_states, target_hidden, cos, sin, Wq, Wk, Wv, Wo,
                 q_norm_w, k_norm_w):
    import ml_dtypes
    bf = ml_dtypes.bfloat16
    c = np.ascontiguousarray
    in_maps = []
    # per-tp staged weights (shared across the 4 batches of each tp group).
    # Everything is laid out SBUF-native: partition dim first, contiguous
    # per partition, so each DMA is one descriptor per partition.
    staged = {}
    for tp in range(TP):
        hsl = slice(tp * HL * HD, (tp + 1) * HL * HD)
        ksl = slice(tp * KVHL * HD, (tp + 1) * KVHL * HD)
        wq_p = Wq[:, hsl].reshape(D, HL, HD)[:, QPERM2, :].reshape(D, HL * HD)
        wkv = np.concatenate([Wk[:, ksl], Wv[:, ksl]], axis=1)
        staged[tp] = {
            # [2048, N] -> [128, 16, N] with partition = d % 128
            "wqP": c(wq_p.reshape(DCH, 128, HL * HD).transpose(1, 0, 2).astype(bf)),
            "wkvP": c(wkv.reshape(DCH, 128, 512).transpose(1, 0, 2).astype(bf)),
            # [1024, 2048] -> [64, 16, 2048] with partition = hd
            "woP": c(Wo[hsl, :].reshape(HL, QL, D).transpose(1, 0, 2).astype(bf)),
        }
    per_b = {}
    for b in range(DP):
        tT = target_hidden[b].T.astype(bf)  # [2048, 4096]
        per_b[b] = {
            # [128, blk, dchunk, 512] with partition = d % 128
            "tgtB": c(tT.reshape(DCH, 128, 8, 512).transpose(1, 2, 0, 3)),
            "hidP": c(hidden_states[b].T.astype(bf).reshape(DCH, 128, QL)
                      .transpose(1, 0, 2)),
            # ctx rope tables: [128, tile, hd] with partition = seq % 128
            "cosP": c(cos[b, :CTX].astype(np.float32).reshape(NT, 128, HD)
                      .transpose(1, 0, 2)),
            "sinP": c(sin[b, :CTX].astype(np.float32).reshape(NT, 128, HD)
                      .transpose(1, 0, 2)),
            "cosd": c(cos[b, CTX:].astype(np.float32)),
            "sind": c(sin[b, CTX:].astype(np.float32)),
        }
    for core in range(8):
        tp, b = core // DP, core % DP
        in_maps.append({
            **per_b[b],
            "qw": c(q_norm_w.reshape(1, HD).astype(np.float32)),
            "kw": c(k_norm_w.reshape(1, HD).astype(np.float32)),
            **staged[tp],
        })
    return in_maps


def kernel(hidden_states, target_hidden, cos, sin, Wq, Wk, Wv, Wo,
           q_norm_w, k_norm_w):
    global LAST_RESULT
    from concourse.bass_utils import run_bass_kernel_spmd

    nc = _get_nc()
    in_maps = make_in_maps(hidden_states, target_hidden, cos, sin,
                           Wq, Wk, Wv, Wo, q_norm_w, k_norm_w)
    trace = os.environ.get("KERNEL_TRACE", "0") == "1"
    res = run_bass_kernel_spmd(nc, in_maps, list(range(8)), trace=trace)
    LAST_RESULT = res
    out = np.zeros((B, QL, D), np.float32)
    for core in range(8):
        tp, b = core // DP, core % DP
        out[b] += res.results[core]["out"]
    return out


# revision 41
# speedup vs baseline: 1.0605x; 1.0605x over previous
"""Trainium2 Bass kernel for DFlashAttention (draft/target cross-attention).

Sharding: TP=2 over heads (16 q heads / 4 kv heads per core) x DP=4 over batch.
Core c = tp*4 + b. Each core computes a partial output [64, 2048] (its 16 heads
through its slice of Wo); the host sums the two TP partials per batch.

Host staging: activations are transposed and cast to bf16 on the host
(tgtT/hidT with the model dim leading), wq columns are permuted so q-head
blocks land j-major for the score matmuls, wk|wv are concatenated.

Per-core pipeline (single fused streaming loop over 32 ctx tiles of 128):
  - K/V projection from resident X^T chunks (bf16, fp32 PSUM accum).
  - RMS-norm + RoPE on K fully on the DVE; rsqrt via the pow ALU op so the
    Act engine only ever runs Exp (no activation-table thrash). The norm
    weights (x8, absorbing the 1/sqrt(mean) scale) are pre-folded into the
    cos/sin tables.
  - K^T materialized via xbar DMA transposes (no PE transposes anywhere).
  - Scores computed transposed (S^T = K Q^T) so probs come out kv-major and
    feed the PV matmul directly; softmax sums ride along as a ones-column in
    V (row 64 of the PV accumulator); normalization happens once at the end.
  - No max-subtraction in softmax (scores provably bounded, see baseline).
  - Output projection in bf16.
"""

import os
import numpy as np
from contextlib import ExitStack

B, QL, CTX, KV, D = 4, 64, 4096, 4160, 2048
H, KVH, HD = 32, 8, 64
TP, DP = 2, 4
HL, KVHL = H // TP, KVH // TP          # 16 q heads, 4 kv heads per core
GL = KVHL                              # 4 GQA groups per core (4 q heads each)
DCH = D // 128                         # 16 contraction chunks
NT = CTX // 128                        # 32 ctx kv tiles
EPS = 1e-6
# q-head permutation: j-major (j = group%2 selects the partition half shared
# with the kv head), so score rhs slices are contiguous. perm[slot] = orig head.
QPERM = [0, 1, 2, 3, 8, 9, 10, 11, 4, 5, 6, 7, 12, 13, 14, 15]
# wq column order: pair-interleaved so each 128-col chunk c of Qb holds
# (QPERM[c], QPERM[8+c]) and a plain 2D xbar transpose of the chunk stacks
# them at partition halves (0:64, 64:128) = exactly QT2[:, c, :].
QPERM2 = [QPERM[8 * (i % 2) + i // 2] for i in range(16)]
# group g -> (partition half j, kv-slot jj, q-slot block offset in QPERM order)
G_J = [g % 2 for g in range(GL)]
G_JJ = [g // 2 for g in range(GL)]
G_QOFF = [8 * (g % 2) + 4 * (g // 2) for g in range(GL)]

_NC = None
LAST_RESULT = None


def _patch_drain_split():
    """The walrus build in this container rejects >1 sync-wait on a TPB_CTRL
    Drain; split the TileContext final-drain waits across single-wait NOPs."""
    import concourse.tile as tile
    import concourse.mybir as mybir
    from concourse.vector_clock import ScopedClock

    if getattr(tile.TileContext, "_drain_split_patched", False):
        return

    def _drain_and_barrier(self, tick_clock, wait_clock):
        nc = self.nc
        drain_inst = nc.sync.drain()
        wait_clock.add_sem_waits(
            drain_inst.ins, ScopedClock({None: tick_clock.global_clock})
        )
        si = drain_inst.ins.sync_info
        if si is not None and len(si.on_wait) > 1:
            waits = list(si.on_wait)
            si.on_wait = []
            for w in waits:
                n = nc.sync.nop(nofuse=True, hint="drain_wait_split")
                n.ins.sync_info = mybir.SyncInfo(on_wait=[w], on_update=[])
        nc.all_engine_barrier()
        assert self.sems is not None
        popped = nc._tile_sem_poison_stack.pop()
        assert popped is self._sem_poison
        nc.clear_and_free_semaphores(list(self.sems.allocated().values()))
        nc.all_engine_barrier()

    tile.TileContext._drain_and_barrier = _drain_and_barrier
    tile.TileContext._drain_split_patched = True


def _split_excess_waits(nc, mybir, limit=1):
    """The walrus build here allows very few sync-waits per instruction;
    hoist excess waits onto single-wait NOPs on the same engine queue."""
    k = 0
    for f in nc.m.functions:
        for b in f.blocks:
            out = []
            for inst in b.instructions:
                si = inst.sync_info
                if si is not None and len(si.on_wait) > limit:
                    waits = list(si.on_wait)
                    si.on_wait = waits[-limit:]
                    for w in waits[:-limit]:
                        n = mybir.InstEventSemaphore(name=f"I-ws{k}", ins=[], outs=[])
                        k += 1
                        n.engine = inst.engine
                        n.sync_info = mybir.SyncInfo(on_wait=[w], on_update=[])
                        nc.register_instruction(n)
                        out.append(n)
                out.append(inst)
            b.instructions[:] = out


def build_nc():
    import concourse.bass as bass
    import concourse.mybir as mybir
    import concourse.tile as tile

    _patch_drain_split()

    f32 = mybir.dt.float32
    f32r = mybir.dt.float32r
    bf16 = mybir.dt.bfloat16
    MUL = mybir.AluOpType.mult
    ADD = mybir.AluOpType.add
    POW = mybir.AluOpType.pow
    AXX = mybir.AxisListType.X
    EXP = mybir.ActivationFunctionType.Exp
    SQ = mybir.ActivationFunctionType.Square
    LN = mybir.ActivationFunctionType.Ln

    nc = bass.Bass("TRN2", target_bir_lowering=False, debug=False, num_devices=8)

    # all inputs are host-staged into SBUF-native layouts (partition dim
    # first, contiguous per partition) so every DMA is one descriptor per
    # partition — the DMA queues are descriptor-bound, not byte-bound.
    tgtB = nc.dram_tensor("tgtB", [128, 8, DCH, 512], bf16, kind="ExternalInput").ap()
    hidP = nc.dram_tensor("hidP", [128, DCH, QL], bf16, kind="ExternalInput").ap()
    cosP = nc.dram_tensor("cosP", [128, NT, HD], f32, kind="ExternalInput").ap()
    sinP = nc.dram_tensor("sinP", [128, NT, HD], f32, kind="ExternalInput").ap()
    cosd = nc.dram_tensor("cosd", [QL, HD], f32, kind="ExternalInput").ap()
    sind = nc.dram_tensor("sind", [QL, HD], f32, kind="ExternalInput").ap()
    wqP = nc.dram_tensor("wqP", [128, DCH, HL * HD], bf16, kind="ExternalInput").ap()
    wkvP = nc.dram_tensor("wkvP", [128, DCH, 512], bf16, kind="ExternalInput").ap()
    woP = nc.dram_tensor("woP", [QL, HL, D], bf16, kind="ExternalInput").ap()
    qw = nc.dram_tensor("qw", [1, HD], f32, kind="ExternalInput").ap()
    kw = nc.dram_tensor("kw", [1, HD], f32, kind="ExternalInput").ap()
    out = nc.dram_tensor("out", [QL, D], f32, kind="ExternalOutput").ap()

    # The x8 in the folded cos/sin tables cancels exactly against
    # riv = (ssq + 64*eps)^-0.5 = rsqrt(mean + eps)/8, so Qb/Kb hold the
    # exact normed+roped vectors and the score scale is just 1/sqrt(hd).
    ESC = 0.125

    with tile.TileContext(nc) as tc, ExitStack() as ctx:
        const = ctx.enter_context(tc.tile_pool(name="const", bufs=1))
        xt_pool = ctx.enter_context(tc.tile_pool(name="xt", bufs=2))
        wq_pool = ctx.enter_context(tc.tile_pool(name="wqp", bufs=2))
        kb_pool = ctx.enter_context(tc.tile_pool(name="kb", bufs=2))
        rs_pool = ctx.enter_context(tc.tile_pool(name="rs", bufs=2))
        stats = ctx.enter_context(tc.tile_pool(name="stats", bufs=2))
        pt_pool = ctx.enter_context(tc.tile_pool(name="pt", bufs=2))

        # ---- persistent SBUF tensors ----
        WKV = const.tile([128, DCH, 512], bf16)   # [d128, chunk, k(256)|v(256)]
        KT2 = const.tile([128, 2, KV], bf16)      # [hd | 2 stacked kv heads, jj, kv]
        VN4 = const.tile([128, NT + 1, GL, HD + 1], bf16)  # v natural + ones col
        QT2 = const.tile([128, 8, QL], bf16)      # [hd | j half, head slot, q]
        WO = const.tile([QL, HL, D], bf16)        # [hd, head, d]
        HT = const.tile([128, DCH, QL], bf16)     # hidden^T chunks
        COSW = const.tile([128, NT, HD], f32)     # ctx cos * kw * 8
        SINSW = const.tile([128, NT, HD], f32)    # ctx sin pattern * rot(kw) * 8
        CDQ = const.tile([QL, HD], f32)
        SDQ = const.tile([QL, HD], f32)
        CDK = const.tile([QL, HD], f32)
        SDK = const.tile([QL, HD], f32)
        ONES = const.tile([1, 128], f32)
        EPSB = const.tile([128, 1], f32)
        NW = const.tile([128, 4, HD], f32)        # bcast kw8, rot(kw8), qw8, rot(qw8)
        KW1 = const.tile([1, HD], f32)
        QW1 = const.tile([1, HD], f32)
        W8 = const.tile([1, 4, HD], f32)
        Qb = const.tile([QL, HL * HD], bf16)
        Kdb = const.tile([QL, KVHL * HD], bf16)
        OTb = const.tile([QL, HL, HD], bf16)
        OUTT = const.tile([QL, D], f32)

        # ---- DMAs: X^T block 0 first (gates the streaming loop), then the
        # small stuff phase 1 needs ----
        XTs = [xt_pool.tile([128, DCH, 512], bf16, tag="xt", name=f"xt{i}")
               for i in range(2)]

        def load_xt(blk, part=None):
            # part=None loads the whole block; part=k loads quarter k — the
            # steady-state loop issues one quarter per tile so XT transfers
            # never monopolize DMA queues at block boundaries.
            dst = XTs[blk % 2]
            parts = range(4) if part is None else [part]
            for k in parts:
                i = 4 * k
                nc.sync.dma_start(dst[:, i:i + 4, :], tgtB[:, blk, i:i + 4, :])

        nc.sync.dma_start(KW1[:], kw)
        nc.sync.dma_start(QW1[:], qw)
        nc.sync.dma_start(HT[:], hidP)
        load_xt(0)
        for half in range(4):
            sl = slice(half * 4, half * 4 + 4)
            nc.sync.dma_start(WKV[:, sl, :], wkvP[:, sl, :])
        nc.sync.dma_start(COSW[:], cosP)
        nc.sync.dma_start(SINSW[:], sinP)
        nc.sync.dma_start(CDQ[:], cosd)
        nc.sync.dma_start(SDQ[:], sind)
        nc.sync.dma_start(CDK[:], cosd)
        nc.sync.dma_start(SDK[:], sind)

        nc.vector.memset(ONES[:], 1.0)
        nc.vector.memset(EPSB[:], float(HD) * EPS)
        nc.vector.memset(VN4[:, :, :, HD:HD + 1], 1.0)

        # ---- norm-weight broadcast tiles (x8 fold) ----
        # W8 rows: [kw*8, rot(kw*8), qw*8, rot(qw*8)]
        nc.vector.tensor_scalar_mul(W8[:, 0, :], KW1[:], 8.0)
        nc.vector.tensor_scalar_mul(W8[:, 2, :], QW1[:], 8.0)
        for r in (0, 2):
            nc.vector.tensor_copy(W8[:, r + 1, 0:32], W8[:, r, 32:64])
            nc.vector.tensor_copy(W8[:, r + 1, 32:64], W8[:, r, 0:32])

        with tc.tile_pool(name="ps1", bufs=1, space="PSUM") as ps1, \
             tc.tile_pool(name="psq", bufs=1, space="PSUM") as psq:
            ps_nw = ps1.tile([128, 4 * HD], f32, tag="nw")
            nc.tensor.matmul(ps_nw[:], lhsT=ONES[:], rhs=W8[:].rearrange("o f d -> o (f d)"),
                             start=True, stop=True)
            nc.vector.tensor_copy(NW[:].rearrange("p f d -> p (f d)"), ps_nw[:])

            # fold weights into rope tables.
            # cos'[i] = cos[i]*w8[i]; sin'[i<32] = -sin[i]*w8[i+32],
            # sin'[i>=32] = sin[i]*w8[i-32]  (rot already applied in NW rows 1/3)
            nwc = NW[:, 0, None, :].broadcast_to([128, NT, HD])
            nc.vector.tensor_mul(COSW[:], COSW[:], nwc)
            nc.vector.scalar_tensor_tensor(
                out=SINSW[:, :, 0:32], in0=SINSW[:, :, 0:32], scalar=-1.0,
                in1=NW[:, 1, None, 0:32].broadcast_to([128, NT, 32]),
                op0=MUL, op1=MUL,
            )
            nc.vector.tensor_mul(
                SINSW[:, :, 32:64], SINSW[:, :, 32:64],
                NW[:, 1, None, 32:64].broadcast_to([128, NT, 32]),
            )
            for ctile, stile, rw in ((CDK, SDK, 0), (CDQ, SDQ, 2)):
                nc.vector.tensor_mul(ctile[:], ctile[:], NW[0:QL, rw, :])
                nc.vector.scalar_tensor_tensor(
                    out=stile[:, 0:32], in0=stile[:, 0:32], scalar=-1.0,
                    in1=NW[0:QL, rw + 1, 0:32], op0=MUL, op1=MUL,
                )
                nc.vector.tensor_mul(stile[:, 32:64], stile[:, 32:64],
                                     NW[0:QL, rw + 1, 32:64])

            def rms_rope_a(src3, P, nh, sins_ap, tag):
                """Independent front half: x^2 on Act (psum reads of the same
                tensor twice are illegal on the DVE) + the rotate-half sin
                products on the DVE. Emitted at block start so the Act/DVE
                queues start the rope before the softmax EXPs queue up."""
                scr = rs_pool.tile([P, nh * HD], f32, tag="rrs", name=tag + "s")
                scr3 = scr[:].rearrange("p (h d) -> p h d", d=HD)
                rot = rs_pool.tile([P, nh * HD], f32, tag="rrr", name=tag + "r")
                rot3 = rot[:].rearrange("p (h d) -> p h d", d=HD)
                mss = stats.tile([P, nh], f32, tag="rrm", name=tag + "m")
                riv = stats.tile([P, nh], f32, tag="rrv", name=tag + "v")
                nc.scalar.activation(scr3[:], src3, SQ)
                s0 = sins_ap[:, None, 0:32].broadcast_to([P, nh, 32])
                s1 = sins_ap[:, None, 32:64].broadcast_to([P, nh, 32])
                nc.vector.tensor_mul(rot3[:, :, 0:32], src3[:, :, 32:64], s0)
                nc.vector.tensor_mul(rot3[:, :, 32:64], src3[:, :, 0:32], s1)
                return (src3, P, nh, scr, scr3, rot, mss, riv)

            def rms_rope_b(st, cos_ap, out_bf, tag):
                """Dependent tail: ssq reduce, riv = exp(-0.5*ln(ssq+64eps))
                (ln/exp/square/copy share one act table — no table reloads),
                cos product, sum, and the riv scale writing bf16."""
                src3, P, nh, scr, scr3, rot, mss, riv = st
                nc.vector.reduce_sum(mss[:, :, None], scr3[:], axis=AXX)
                lg = stats.tile([P, nh], f32, tag="rrl", name=tag + "l")
                nc.scalar.activation(lg[:], mss[:], LN, bias=EPSB[0:P, :], scale=1.0)
                nc.scalar.activation(riv[:], lg[:], EXP, scale=-0.5)
                cb = cos_ap[:, None, :].broadcast_to([P, nh, HD])
                nc.vector.tensor_mul(scr3[:], src3, cb)
                nc.vector.tensor_add(scr[:], scr[:], rot[:])
                ob3 = out_bf.rearrange("p (h d) -> p h d", d=HD)
                nc.vector.tensor_mul(ob3[:], scr3[:], riv[:, :, None].broadcast_to([P, nh, HD]))

            def rms_rope(src3, P, nh, cos_ap, sins_ap, out_bf, tag):
                st = rms_rope_a(src3, P, nh, sins_ap, tag)
                rms_rope_b(st, cos_ap, out_bf, tag)

            # ---- phase 1: q + draft k/v ----
            ps_q = [psq.tile([QL, 512], f32, tag=f"q{c}", name=f"ps_q{c}")
                    for c in range(2)]
            ps_kvd = psq.tile([QL, 512], f32, tag="kvd")
            for i in range(DCH):
                if i % 4 == 0:
                    wqt = wq_pool.tile([128, 4, HL * HD], bf16, tag="w")
                    nc.sync.dma_start(wqt[:], wqP[:, i:i + 4, :])
                for c in range(2):
                    nc.tensor.matmul(
                        ps_q[c][:], lhsT=HT[:, i, :],
                        rhs=wqt[:, i % 4, c * 512:(c + 1) * 512],
                        start=(i == 0), stop=(i == DCH - 1),
                    )
                nc.tensor.matmul(
                    ps_kvd[:], lhsT=HT[:, i, :], rhs=WKV[:, i, :],
                    start=(i == 0), stop=(i == DCH - 1),
                )

            for c in range(4):
                rms_rope(ps_q[c // 2][:, (c % 2) * 256:(c % 2) * 256 + 256]
                         .rearrange("p (h d) -> p h d", d=HD), QL, KVHL,
                         CDQ[:], SDQ[:], Qb[:, c * 256:(c + 1) * 256], "q")
            # monolithic 3D-out xbar transposes are broken on HW; per-128-col
            # 2D transposes with the pair-interleaved wq column order land
            # each chunk's head pair at partition halves (0:64, 64:128).
            for cch in range(8):
                nc.sync.dma_start_transpose(
                    QT2[:, cch, :], Qb[:, cch * 128:(cch + 1) * 128]
                )

            rms_rope(ps_kvd[:, 0:256].rearrange("p (h d) -> p h d", d=HD), QL, KVHL,
                     CDK[:], SDK[:], Kdb[:], "kd")
            nc.scalar.copy(
                VN4[0:QL, NT, :, 0:HD],
                ps_kvd[:, 256:512].rearrange("p (h d) -> p h d", d=HD),
            )
            nc.sync.dma_start_transpose(KT2[:, 0, CTX:KV], Kdb[:, 0:128])
            nc.sync.dma_start_transpose(KT2[:, 1, CTX:KV], Kdb[:, 128:256])

        # ---- fused streaming loop over ctx tiles ----
        ps_o_cm = tc.tile_pool(name="pso", bufs=1, space="PSUM")
        ps_kv_cm = tc.tile_pool(name="pskv", bufs=2, space="PSUM")
        ps_s_cm = tc.tile_pool(name="pss", bufs=1, space="PSUM")
        ps_op = ps_o_cm.__enter__()
        ps_kvp = ps_kv_cm.__enter__()
        ps_sp = ps_s_cm.__enter__()
        ps_o = [ps_op.tile([HD + 1, 4 * QL], f32, tag=f"o{g}", name=f"ps_o{g}")
                for g in range(GL)]

        def scores_block(t, cw):
            """Emit scores + exp for kv tile t ([c0, c0+cw))."""
            c0 = t * 128
            pss = []
            for j in range(2):
                ps_s = ps_sp.tile([128, 512], f32, tag=f"s{j}", name=f"ps_s{t}_{j}")
                for a in range(2):
                    g = j + 2 * a
                    nc.tensor.matmul(
                        ps_s[0:cw, a * 256:(a + 1) * 256],
                        lhsT=KT2[64 * j:64 * j + 64, G_JJ[g], c0:c0 + cw],
                        rhs=QT2[64 * j:64 * j + 64, 4 * G_JJ[g]:4 * G_JJ[g] + 4, :],
                        start=True, stop=True,
                    )
                pss.append(ps_s)
            pts = []
            for j in range(2):
                pt = pt_pool.tile([128, 512], bf16, tag=f"p{j}", name=f"pt{t}_{j}")
                nc.scalar.activation(pt[0:cw, :], pss[j][0:cw, :], EXP, scale=ESC)
                pts.append(pt)
            return pts

        def pv_block(t, cw, pts, start, stop):
            for j in range(2):
                for a in range(2):
                    g = j + 2 * a
                    nc.tensor.matmul(
                        ps_o[g][:],
                        lhsT=VN4[0:cw, t, g, :],
                        rhs=pts[j][0:cw, a * 256:(a + 1) * 256],
                        start=start, stop=stop,
                    )

        def rope_k_a(t):
            ps_kv = kv_tiles[t % 2]
            src3 = ps_kv[:, 0:256].rearrange("p (h d) -> p h d", d=HD)
            return rms_rope_a(src3, 128, KVHL, SINSW[:, t, :], "k")

        def rope_k_b(t, st):
            ps_kv = kv_tiles[t % 2]
            kb = kb_pool.tile([128, KVHL * HD], bf16, tag="kb", name=f"kb{t}")
            rms_rope_b(st, COSW[:, t, :], kb[:], "k")
            nc.sync.dma_start_transpose(KT2[:, 0, t * 128:(t + 1) * 128], kb[:, 0:128])
            nc.sync.dma_start_transpose(KT2[:, 1, t * 128:(t + 1) * 128], kb[:, 128:256])
            nc.vector.tensor_copy(
                VN4[:, t, :, 0:HD],
                ps_kv[:, 256:512].rearrange("p (h d) -> p h d", d=HD),
            )

        # scores/PV lag the projection by LAG tiles: the rope chain (Act/DVE,
        # ~4us latency incl. queue waits) plus the K^T xbar DMA (~2us
        # init+transfer) must complete before the scores read KT2 — a short
        # lag stalls the PE every tile and resets its p-state ramp.
        LAG = 4
        kv_tiles = [None, None]
        rope_st = None
        for t in range(NT):
            if t // 4 + 1 < 8:
                load_xt(t // 4 + 1, part=t % 4)
            if 8 <= t < 16:
                hh = 2 * (t - 8)
                nc.sync.dma_start(WO[:, hh:hh + 2, :], woP[:, hh:hh + 2, :])
            if t >= 1:
                rope_st = rope_k_a(t - 1)
            pts = None
            if t >= LAG:
                pts = scores_block(t - LAG, 128)
            ps_kv = ps_kvp.tile([128, 512], f32, tag="kv", name=f"ps_kv{t}")
            kv_tiles[t % 2] = ps_kv
            xt = XTs[(t // 4) % 2]
            for i in range(DCH):
                nc.tensor.matmul(
                    ps_kv[:], lhsT=xt[:, i, (t % 4) * 128:(t % 4) * 128 + 128],
                    rhs=WKV[:, i, :],
                    start=(i == 0), stop=(i == DCH - 1),
                )
            if t >= LAG:
                pv_block(t - LAG, 128, pts, start=(t - LAG == 0), stop=False)
            if t >= 1:
                rope_k_b(t - 1, rope_st)

        # drain the pipeline: tiles NT-LAG..NT-1, then the draft tail
        st31 = rope_k_a(NT - 1)
        rope_k_b(NT - 1, st31)
        for tt in range(NT - LAG, NT):
            pts = scores_block(tt, 128)
            pv_block(tt, 128, pts, start=False, stop=False)
        pts = scores_block(NT, QL)
        pv_block(NT, QL, pts, start=False, stop=True)

        ps_s_cm.__exit__(None, None, None)
        ps_kv_cm.__exit__(None, None, None)

        # ---- normalize attention output ----
        with tc.tile_pool(name="psn", bufs=2, space="PSUM") as psn:
            for g in range(GL):
                li = stats.tile([1, 4 * QL], f32, tag="li", name=f"li{g}", bufs=1)
                nc.vector.reciprocal(li[:], ps_o[g][HD:HD + 1, :])
                ps_lib = psn.tile([HD, 4 * QL], f32, tag="lib", name=f"lib{g}")
                nc.tensor.matmul(ps_lib[:], lhsT=ONES[:, 0:HD], rhs=li[:],
                                 start=True, stop=True)
                lib = stats.tile([HD, 4 * QL], f32, tag="libs", name=f"libs{g}", bufs=1)
                nc.vector.tensor_copy(lib[:], ps_lib[:])
                off = G_QOFF[g]
                nc.vector.tensor_mul(
                    OTb[:, off:off + 4, :],
                    ps_o[g][0:HD, :].rearrange("p (h q) -> p h q", q=QL),
                    lib[:].rearrange("p (h q) -> p h q", q=QL),
                )

        ps_o_cm.__exit__(None, None, None)

        # ---- output projection (bf16) ----
        with tc.tile_pool(name="ps4", bufs=1, space="PSUM") as ps4:
            accs = [ps4.tile([QL, 512], f32, tag=f"a{c}", name=f"acc{c}")
                    for c in range(4)]
            for hp in range(HL):
                h = QPERM[hp]
                for cc in range(4):
                    nc.tensor.matmul(
                        accs[cc][:],
                        lhsT=OTb[:, hp, :],
                        rhs=WO[:, h, cc * 512:(cc + 1) * 512],
                        start=(hp == 0), stop=(hp == HL - 1),
                    )
            for cc in range(4):
                if cc % 2 == 0:
                    nc.vector.tensor_copy(OUTT[:, cc * 512:(cc + 1) * 512], accs[cc][:])
                else:
                    nc.scalar.copy(OUTT[:, cc * 512:(cc + 1) * 512], accs[cc][:])
        nc.sync.dma_start(out, OUTT[:])

    _split_excess_waits(nc, mybir)
    return nc


def _get_nc():
    global _NC
    if _NC is None:
        _NC = build_nc()
    return _NC


def make_in_maps(hidden_states, target_hidden, cos, sin, Wq, Wk, Wv, Wo,
                 q_norm_w, k_norm_w):
    import ml_dtypes
    bf = ml_dtypes.bfloat16
    c = np.ascontiguousarray
    in_maps = []
    # per-tp staged weights (shared across the 4 batches of each tp group).
    # Everything is laid out SBUF-native: partition dim first, contiguous
    # per partition, so each DMA is one descriptor per partition.
    staged = {}
    for tp in range(TP):
        hsl = slice(tp * HL * HD, (tp + 1) * HL * HD)
        ksl = slice(tp * KVHL * HD, (tp + 1) * KVHL * HD)
        wq_p = Wq[:, hsl].reshape(D, HL, HD)[:, QPERM2, :].reshape(D, HL * HD)
        wkv = np.concatenate([Wk[:, ksl], Wv[:, ksl]], axis=1)
        staged[tp] = {
            # [2048, N] -> [128, 16, N] with partition = d % 128
            "wqP": c(wq_p.reshape(DCH, 128, HL * HD).transpose(1, 0, 2).astype(bf)),
            "wkvP": c(wkv.reshape(DCH, 128, 512).transpose(1, 0, 2).astype(bf)),
            # [1024, 2048] -> [64, 16, 2048] with partition = hd
            "woP": c(Wo[hsl, :].reshape(HL, QL, D).transpose(1, 0, 2).astype(bf)),
        }
    per_b = {}
    for b in range(DP):
        tT = target_hidden[b].T.astype(bf)  # [2048, 4096]
        per_b[b] = {
            # [128, blk, dchunk, 512] with partition = d % 128
            "tgtB": c(tT.reshape(DCH, 128, 8, 512).transpose(1, 2, 0, 3)),
            "hidP": c(hidden_states[b].T.astype(bf).reshape(DCH, 128, QL)
                      .transpose(1, 0, 2)),
            # ctx rope tables: [128, tile, hd] with partition = seq % 128
            "cosP": c(cos[b, :CTX].astype(np.float32).reshape(NT, 128, HD)
                      .transpose(1, 0, 2)),
            "sinP": c(sin[b, :CTX].astype(np.float32).reshape(NT, 128, HD)
                      .transpose(1, 0, 2)),
            "cosd": c(cos[b, CTX:].astype(np.float32)),
            "sind": c(sin[b, CTX:].astype(np.float32)),
        }
    for core in range(8):
        tp, b = core // DP, core % DP
        in_maps.append({
            **per_b[b],
            "qw": c(q_norm_w.reshape(1, HD).astype(np.float32)),
            "kw": c(k_norm_w.reshape(1, HD).astype(np.float32)),
            **staged[tp],
        })
    return in_maps


def kernel(hidden_states, target_hidden, cos, sin, Wq, Wk, Wv, Wo,
           q_norm_w, k_norm_w):
    global LAST_RESULT
    from concourse.bass_utils import run_bass_kernel_spmd

    nc = _get_nc()
    in_maps = make_in_maps(hidden_states, target_hidden, cos, sin,
                           Wq, Wk, Wv, Wo, q_norm_w, k_norm_w)
    trace = os.environ.get("KERNEL_TRACE", "0") == "1"
    res = run_bass_kernel_spmd(nc, in_maps, list(range(8)), trace=trace)
    LAST_RESULT = res
    out = np.zeros((B, QL, D), np.float32)
    for core in range(8):
        tp, b = core // DP, core % DP
        out[b] += res.results[core]["out"]
    return out
